# revision 10
# baseline (speedup 1.0000x reference)
import sys, os
sys.path.insert(0, "/opt/trn_rl_repo")
import numpy as np

NCORES = 8
B = 128           # block size
NBT = 32          # matrix block-rows (4096/128)
WSL = 640         # slab width: 512 matrix cols + 128 border block
NS_ITERS = int(os.environ.get('NS_ITERS', '3'))
SSTEPS = int(os.environ.get('SSTEPS', '8'))
NOCC = os.environ.get('NOCC', '0') == '1'
CONST = 0.5 * 2048 * float(np.log(2 * np.pi))

def f_off(k):
    """global block index -> column offset in the gathered row [128, 4096]."""
    if k < 16:
        return 512 * (k // 2) + 128 * (k % 2)
    return 512 * ((k - 16) // 2) + 256 + 128 * ((k - 16) % 2)


_NC_CACHE = None

def build():
    global _NC_CACHE
    if _NC_CACHE is not None:
        return _NC_CACHE
    import concourse.bass as bass
    import concourse.bacc as bacc
    import concourse.tile as tile
    import concourse.mybir as mybir
    dt = mybir.dt
    A_ = mybir.AluOpType
    AF = mybir.ActivationFunctionType

    nc = bacc.Bacc("TRN2", target_bir_lowering=False, debug=False, num_devices=NCORES)
    def din(name, shape):
        return nc.dram_tensor(name, shape, dt.float32, kind="ExternalInput").ap()
    xsT = din("xsT", [256, 256]);  xtT = din("xtT", [256, 256])
    bsT = din("bsT", [256, 512]);  btT = din("btT", [256, 512])
    vaS = din("vaS", [256, 1]);    vaT = din("vaT", [256, 1])
    vcS = din("vcS", [256, 1]);    vcT = din("vcT", [256, 1])
    bS = din("bS", [1, 1]);        bT = din("bT", [1, 1])
    noiseS = din("noiseS", [1, 1]); noiseT = din("noiseT", [1, 1])
    Kb_in = din("Kb", [512, 512])
    khad = din("khad", [4096, 512])
    khp0 = din("khp0", [512, 4096])
    ybcol = din("ybcol", [4224, 1])
    yrow = din("yrow", [1, WSL])
    ones_row = din("ones_row", [1, 128])
    Imask_in = din("Imask", [128, 128])
    onescol = din("onescol", [128, 1])
    nsel0 = din("nsel0", [128, 16]); nsel1 = din("nsel1", [128, 16])
    nsel2 = din("nsel2", [128, 16]); nsel3 = din("nsel3", [128, 16])
    loss_out = nc.dram_tensor("loss", [1, 1], dt.float32, kind="ExternalOutput").ap()
    DGDUMP = os.environ.get('DGDUMP', '0') == '1'
    dg_out = nc.dram_tensor("dgdump", [128, 32], dt.float32, kind="ExternalOutput").ap() if DGDUMP else None
    ar4_out = nc.dram_tensor("ar4dump", [128, 512], dt.float32, kind="ExternalOutput").ap() if DGDUMP else None
    gr0_out = nc.dram_tensor("gr0dump", [128, 4096], dt.bfloat16, kind="ExternalOutput").ap() if DGDUMP else None

    with tile.TileContext(nc) as tc:
        with tc.tile_pool(name="pers", bufs=1) as P, \
             tc.tile_pool(name="gpool", bufs=1) as G, \
             tc.tile_pool(name="strm", bufs=2) as S, \
             tc.tile_pool(name="pw", bufs=2, space="PSUM") as PW, \
             tc.tile_pool(name="psq", bufs=2, space="PSUM") as PQ, \
             tc.tile_pool(name="pt", bufs=1, space="PSUM") as PT, \
             tc.tile_pool(name="dram", bufs=2, space="DRAM") as DR:

            # ---------- constants into SBUF ----------
            Imask = P.tile([128, 128], dt.float32, tag="Imask", name="Imask")
            nc.sync.dma_start(out=Imask[:, :], in_=Imask_in[:, :])
            onesr = P.tile([1, 128], dt.float32, tag="onesr", name="onesr")
            nc.sync.dma_start(out=onesr[:, :], in_=ones_row[:, :])
            onesc = P.tile([128, 1], dt.float32, tag="onesc", name="onesc")
            nc.sync.dma_start(out=onesc[:, :], in_=onescol[:, :])

            # ---------- A slab ----------
            Arow = [P.tile([128, WSL], dt.float32, tag=f"A{i}", name=f"A{i}")
                    for i in range(NBT + 1)]

            # ---------- grow tiles (gathered panel rows), Vfull ----------
            grow = [G.tile([128, 4096], dt.bfloat16, tag=f"grow{j}", name=f"grow{j}", bufs=1)
                    for j in range(4)]

            # ---------- stage 1: w build ----------
            st1 = tc.tile_pool(name="st1", bufs=1)
            P1 = st1.__enter__()
            wlocb = [P1.tile([128, 512], dt.bfloat16, tag=f"wlocb{q}", name=f"wlocb{q}") for q in range(4)]

            def build_w(xT, bT_, va_in, vc_in, bias, m_off, nm):
                # load inputs
                xt = [S.tile([128, 256], dt.float32, tag="xt", name="xt") for _ in range(2)]
                nc.sync.dma_start(out=xt[0][:, :], in_=xT[0:128, :])
                nc.sync.dma_start(out=xt[1][:, :], in_=xT[128:256, :])
                bt_ = [S.tile([128, 512], dt.float32, tag="bt", name="bt") for _ in range(2)]
                nc.sync.dma_start(out=bt_[0][:, :], in_=bT_[0:128, :])
                nc.sync.dma_start(out=bt_[1][:, :], in_=bT_[128:256, :])
                va = [S.tile([128, 1], dt.float32, tag="va", name="va") for _ in range(2)]
                nc.sync.dma_start(out=va[0][:, :], in_=va_in[0:128, :])
                nc.sync.dma_start(out=va[1][:, :], in_=va_in[128:256, :])
                vc = [S.tile([128, 1], dt.float32, tag="vc", name="vc") for _ in range(2)]
                nc.sync.dma_start(out=vc[0][:, :], in_=vc_in[0:128, :])
                nc.sync.dma_start(out=vc[1][:, :], in_=vc_in[128:256, :])
                bia = S.tile([1, 1], dt.float32, tag="bia", name="bia")
                nc.sync.dma_start(out=bia[:, :], in_=bias[:, :])

                # crow = vc^T @ base^T + bias : [1, 512]  (vc = Wf^T @ Ww[FEAT:])
                pc = PT.tile([1, 512], dt.float32, tag="pt", name="pt")
                for kc in range(2):
                    nc.tensor.matmul(pc[:, :], vc[kc][:, :], bt_[kc][:, :],
                                     start=(kc == 0), stop=(kc == 1))
                crow = S.tile([1, 512], dt.float32, tag="crow", name="crow")
                nc.scalar.activation(out=crow[:, :], in_=pc[:, :], func=AF.Identity,
                                     bias=bia[:, :], scale=1.0)
                # bcast crow to 128 partitions
                pb = PW.tile([128, 512], dt.float32, tag="pw", name="pw")
                nc.tensor.matmul(pb[:, :], onesr[:, :], crow[:, :], start=True, stop=True)

                for r in range(2):
                    # a_r = x_rows . va  (va = Wf^T @ Ww[:FEAT])
                    pf = PQ.tile([128, 1], dt.float32, tag="psq", name="psq")
                    for kc in range(2):
                        nc.tensor.matmul(pf[:, :], xt[kc][:, 128 * r:128 * r + 128],
                                         va[kc][:, :], start=(kc == 0), stop=(kc == 1))
                    a_r = S.tile([128, 1], dt.float32, tag="a_r", name="a_r")
                    nc.vector.tensor_copy(out=a_r[:, :], in_=pf[:, :])
                    raw = S.tile([128, 512], dt.float32, tag="raw", name="raw", bufs=1)
                    nc.vector.tensor_scalar(out=raw[:, :], in0=pb[:, :], scalar1=a_r[:, :],
                                            scalar2=None, op0=A_.add)
                    mn = S.tile([128, 512], dt.float32, tag="mn", name="mn", bufs=1)
                    nc.vector.tensor_scalar(out=mn[:, :], in0=raw[:, :], scalar1=0.0,
                                            scalar2=None, op0=A_.min)
                    ex = S.tile([128, 512], dt.float32, tag="ex", name="ex", bufs=1)
                    nc.scalar.activation(out=ex[:, :], in_=mn[:, :], func=AF.Exp)
                    mx = S.tile([128, 512], dt.float32, tag="mx", name="mx", bufs=1)
                    nc.vector.tensor_scalar(out=mx[:, :], in0=raw[:, :], scalar1=0.0,
                                            scalar2=None, op0=A_.max)
                    w_r = S.tile([128, 512], dt.float32, tag="w_r", name="w_r", bufs=1)
                    nc.vector.scalar_tensor_tensor(out=w_r[:, :], in0=ex[:, :], scalar=-1.0,
                                                   in1=mx[:, :], op0=A_.add, op1=A_.add)
                    sm = S.tile([128, 1], dt.float32, tag="sm", name="sm")
                    nc.vector.tensor_reduce(out=sm[:, :], in_=w_r[:, :],
                                            axis=mybir.AxisListType.X, op=A_.add)
                    rec = S.tile([128, 1], dt.float32, tag="rec", name="rec")
                    nc.vector.reciprocal(out=rec[:, :], in_=sm[:, :])
                    nc.vector.tensor_scalar(out=w_r[:, :], in0=w_r[:, :], scalar1=rec[:, :],
                                            scalar2=512.0, op0=A_.mult, op1=A_.mult)
                    # transpose 128x128 chunks into wlocb[q][:, m_off+128r : +128]
                    for q in range(4):
                        ptr = PQ.tile([128, 128], dt.float32, tag="psq", name="psq")
                        nc.tensor.matmul(ptr[:, :], w_r[:, 128 * q:128 * q + 128], Imask[:, :], start=True, stop=True)
                        nc.vector.tensor_copy(
                            out=wlocb[q][:, m_off + 128 * r: m_off + 128 * r + 128],
                            in_=ptr[:, :])

            build_w(xsT, bsT, vaS, vcS, bS, 0, "s")
            build_w(xtT, btT, vaT, vcT, bT, 256, "t")

            # ---------- single merged w^T AllGather (bf16) ----------
            gin = DR.tile([128, 2048], dt.bfloat16, tag="gin", name="gin")
            gout = DR.tile([1024, 2048], dt.bfloat16, tag="gout", name="gout",
                           addr_space="Local" if NOCC else "Shared")
            for q in range(4):
                nc.gpsimd.dma_start(out=gin[:, 512 * q:512 * q + 512], in_=wlocb[q][:, :])
            if NOCC:
                for c in range(NCORES):
                    nc.gpsimd.dma_start(out=gout[128 * c:128 * c + 128, :], in_=gin[:, :])
            else:
                nc.gpsimd.collective_compute("AllGather", A_.bypass,
                                             replica_groups=[list(range(NCORES))],
                                             ins=[gin[:, :].opt()], outs=[gout[:, :].opt()])
            def wTg_slice(q, x0, wid, tagn):
                """DMA wTg[q][:, x0:x0+wid] (gathered w^T slice) from gout."""
                c, off = x0 // 512, x0 % 512
                t = S.tile([128, wid], dt.bfloat16, tag=tagn, name=tagn)
                nc.sync.dma_start(out=t[:, :],
                                  in_=gout[128 * c:128 * c + 128,
                                           512 * q + off:512 * q + off + wid])
                return t

            # ---------- Kb ----------
            Kbb = [P1.tile([128, 512], dt.bfloat16, tag=f"Kbb{j}", name=f"Kbb{j}") for j in range(4)]
            for j in range(4):
                kbt = S.tile([128, 512], dt.float32, tag="kbt", name="kbt")
                nc.sync.dma_start(out=kbt[:, :], in_=Kb_in[128 * j:128 * j + 128, :])
                nc.vector.tensor_copy(out=Kbb[j][:, :], in_=kbt[:, :])

            # ---------- wKloc = (Kb @ w_own^T) chunks, bf16 [128, 512] x4 ----------
            wKloc = [P1.tile([128, 512], dt.bfloat16, tag=f"wKloc{q}", name=f"wKloc{q}") for q in range(4)]
            for q in range(4):
                pm = PW.tile([128, 512], dt.float32, tag="pw", name="pw")
                for jt in range(4):
                    nc.tensor.matmul(pm[:, :], Kbb[jt][:, 128 * q:128 * q + 128],
                                     wlocb[jt][:, :], start=(jt == 0), stop=(jt == 3))
                nc.vector.tensor_copy(out=wKloc[q][:, :], in_=pm[:, :])

            # ---------- wKp = (Kb @ w_panel0^T) chunks, bf16 [128,128] x4kk x4q ----------
            wKp = [[P1.tile([128, 128], dt.bfloat16, tag=f"wKp{kk}_{q}", name=f"wKp{kk}_{q}")
                    for q in range(4)] for kk in range(4)]
            for kk in range(4):
                fo = f_off(kk)
                wfo = [wTg_slice(jt, fo, 128, f"wfo{jt}") for jt in range(4)]
                for q in range(4):
                    pm = PQ.tile([128, 128], dt.float32, tag="psq", name="psq")
                    for jt in range(4):
                        nc.tensor.matmul(pm[:, :], Kbb[jt][:, 128 * q:128 * q + 128],
                                         wfo[jt][:, :], start=(jt == 0), stop=(jt == 3))
                    nc.vector.tensor_copy(out=wKp[kk][q][:, :], in_=pm[:, :])

            # ---------- noise prep ----------
            def noise_bcast(nin, tagp):
                cl = S.tile([1, 1], dt.float32, tag=f"cl{tagp}", name=f"cl{tagp}")
                nin_sb = S.tile([1, 1], dt.float32, tag=f"ni{tagp}", name=f"ni{tagp}")
                nc.sync.dma_start(out=nin_sb[:, :], in_=nin[:, :])
                nc.vector.tensor_scalar(out=cl[:, :], in0=nin_sb[:, :], scalar1=1e-5,
                                        scalar2=1.0, op0=A_.max, op1=A_.min)
                pn = PT.tile([128, 1], dt.float32, tag="pt", name="pt")
                nc.tensor.matmul(pn[:, :], onesr[:, :], cl[:, :], start=True, stop=True)
                nb = P.tile([128, 1], dt.float32, tag=f"nb{tagp}", name=f"nb{tagp}")
                nc.vector.tensor_copy(out=nb[:, :], in_=pn[:, :])
                return nb
            nbS = noise_bcast(noiseS, "S")
            nbT = noise_bcast(noiseT, "T")
            nsc = []
            for idx, (nsin, nb) in enumerate([(nsel0, nbS), (nsel1, nbS), (nsel2, nbT), (nsel3, nbT)]):
                t = S.tile([128, 16], dt.float32, tag=f"nselt{idx}", name=f"nselt{idx}")
                nc.sync.dma_start(out=t[:, :], in_=nsin[:, :])
                sc = P.tile([128, 16], dt.float32, tag=f"nsc{idx}", name=f"nsc{idx}")
                nc.vector.tensor_scalar(out=sc[:, :], in0=t[:, :], scalar1=nb[:, :],
                                        scalar2=None, op0=A_.mult)
                nsc.append(sc)

            # ---------- grow0: panel-0 gathered rows built locally ----------
            for n in range(8):
                wn = [wTg_slice(q, 512 * n, 512, f"wn{q}") for q in range(4)]
                for kk in range(4):
                    pm = PW.tile([128, 512], dt.float32, tag="pw", name="pw")
                    for q in range(4):
                        nc.tensor.matmul(pm[:, :], wKp[kk][q][:, :], wn[q][:, :],
                                         start=(q == 0), stop=(q == 3))
                    khp = S.tile([128, 512], dt.float32, tag="khp", name="khp")
                    nc.sync.dma_start(out=khp[:, :], in_=khp0[128 * kk:128 * kk + 128,
                                                           512 * n:512 * n + 512])
                    nc.vector.scalar_tensor_tensor(out=grow[kk][:, 512 * n:512 * n + 512],
                                                   in0=pm[:, :], scalar=1.0,
                                                   in1=khp[:, :], op0=A_.mult, op1=A_.mult)
            for kk in range(4):
                fo = f_off(kk)
                nc.vector.scalar_tensor_tensor(out=grow[kk][:, fo:fo + 128],
                                               in0=Imask[:, :], scalar=nbS[:, 0:1],
                                               in1=grow[kk][:, fo:fo + 128],
                                               op0=A_.mult, op1=A_.add)

            # ---------- A build ----------
            nc.vector.memset(Arow[NBT][:, :], 0.0)
            nc.sync.dma_start(out=Arow[NBT][0:1, :], in_=yrow[:, :])
            for i in range(NBT):
                wi = [wTg_slice(q, f_off(i), 128, f"wi{q}") for q in range(4)]
                pm = PW.tile([128, 512], dt.float32, tag="pw", name="pw")
                for q in range(4):
                    nc.tensor.matmul(pm[:, :], wi[q][:, :],
                                     wKloc[q][:, :], start=(q == 0), stop=(q == 3))
                kh = S.tile([128, 512], dt.float32, tag="kh", name="kh")
                nc.sync.dma_start(out=kh[:, :], in_=khad[128 * i:128 * i + 128, :])
                nc.vector.scalar_tensor_tensor(out=Arow[i][:, 0:512], in0=pm[:, :], scalar=1.0,
                                               in1=kh[:, :], op0=A_.mult, op1=A_.mult)
                if i < 16:
                    nc.vector.scalar_tensor_tensor(
                        out=Arow[i][:, 0:128], in0=Imask[:, :], scalar=nsc[0][:, i:i + 1],
                        in1=Arow[i][:, 0:128], op0=A_.mult, op1=A_.add)
                    nc.vector.scalar_tensor_tensor(
                        out=Arow[i][:, 128:256], in0=Imask[:, :], scalar=nsc[1][:, i:i + 1],
                        in1=Arow[i][:, 128:256], op0=A_.mult, op1=A_.add)
                else:
                    nc.vector.scalar_tensor_tensor(
                        out=Arow[i][:, 256:384], in0=Imask[:, :], scalar=nsc[2][:, i - 16:i - 15],
                        in1=Arow[i][:, 256:384], op0=A_.mult, op1=A_.add)
                    nc.vector.scalar_tensor_tensor(
                        out=Arow[i][:, 384:512], in0=Imask[:, :], scalar=nsc[3][:, i - 16:i - 15],
                        in1=Arow[i][:, 384:512], op0=A_.mult, op1=A_.add)
                nc.sync.dma_start(out=Arow[i][:, 512:513], in_=ybcol[128 * i:128 * i + 128, :])
                nc.vector.memset(Arow[i][:, 513:WSL], 0.0)

            st1.__exit__(None, None, None)

            # ---------- stage 2: superstep elimination ----------
            st2 = tc.tile_pool(name="st2", bufs=1)
            P2 = st2.__enter__()
            Vfull1 = P2.tile([128, 4096], dt.bfloat16, tag="Vf", name="Vf", bufs=1)
            Vfull = [Vfull1, Vfull1]
            rowbb = [P2.tile([128, 128], dt.bfloat16, tag=f"rowbb{j}", name=f"rowbb{j}") for j in range(4)]
            Vloc = [P2.tile([128, 512], dt.bfloat16, tag=f"Vloc{j}", name=f"Vloc{j}") for j in range(4)]
            Vbord = [P2.tile([128, 128], dt.bfloat16, tag=f"Vbord{j}", name=f"Vbord{j}") for j in range(4)]
            ld_acc = [P2.tile([128, 1], dt.float32, tag=f"ld{j}", name=f"ld{j}") for j in range(2)]
            nc.vector.memset(ld_acc[0][:, :], 0.0)
            ld_cur = 0
            cmid = P2.tile([1, 1], dt.float32, tag="cmid", name="cmid")
            cend = P2.tile([1, 1], dt.float32, tag="cend", name="cend")
            nc.vector.memset(cmid[:, :], 0.0)
            nc.vector.memset(cend[:, :], 0.0)

            def batch_update(rows, VL, VB, RB, GR):
                """A[r] -= sum_kk G_r_kk^T @ [VL_kk | VB_kk] for listed rows."""
                for r in rows:
                    pu = PW.tile([128, WSL], dt.float32, tag="pw", name="pw")
                    for k2 in range(4):
                        lhsT = GR[k2][:, f_off(r):f_off(r) + 128] if r < NBT else RB[k2][:, :]
                        nc.tensor.matmul(pu[:, 0:512], lhsT, VL[k2][:, :],
                                         start=(k2 == 0), stop=(k2 == 3))
                    for k2 in range(4):
                        lhsT = GR[k2][:, f_off(r):f_off(r) + 128] if r < NBT else RB[k2][:, :]
                        nc.tensor.matmul(pu[:, 512:WSL], lhsT, VB[k2][:, :],
                                         start=(k2 == 0), stop=(k2 == 3))
                    nc.vector.scalar_tensor_tensor(out=Arow[r][:, 0:512],
                                                   in0=Arow[r][:, 0:512], scalar=1.0,
                                                   in1=pu[:, 0:512], op0=A_.mult, op1=A_.subtract)
                    nc.vector.scalar_tensor_tensor(out=Arow[r][:, 512:WSL],
                                                   in0=Arow[r][:, 512:WSL], scalar=1.0,
                                                   in1=pu[:, 512:WSL], op0=A_.mult, op1=A_.subtract)

            for s in range(SSTEPS):
                # ---- panel factorization (pivots k = 4s+kk) ----
                for kk in range(4):
                    k = 4 * s + kk
                    fo = f_off(k)
                    if DGDUMP and s == 1 and kk == 0:
                        nc.sync.dma_start(out=ar4_out[:, :], in_=Arow[4][:, 0:512])
                        nc.sync.dma_start(out=gr0_out[:, :], in_=grow[0][:, :])
                    # bf16 casts of pivot row local slab + border
                    nc.vector.tensor_copy(out=rowbb[kk][:, :], in_=Arow[k][:, 512:WSL])
                    rloc = S.tile([128, 512], dt.bfloat16, tag="rloc", name="rloc")
                    nc.vector.tensor_copy(out=rloc[:, :], in_=Arow[k][:, 0:512])
                    # D_k (f32) from gathered row
                    Dk = S.tile([128, 128], dt.float32, tag="Dk", name="Dk")
                    nc.vector.tensor_copy(out=Dk[:, :], in_=grow[kk][:, fo:fo + 128])
                    scr = S.tile([128, 128], dt.float32, tag="scrD", name="scrD")
                    nc.vector.scalar_tensor_tensor(out=scr[:, :], in0=Dk[:, :], scalar=1.0,
                                                   in1=Imask[:, :], op0=A_.mult, op1=A_.mult)
                    dg = S.tile([128, 1], dt.float32, tag="dg", name="dg")
                    nc.vector.tensor_reduce(out=dg[:, :], in_=scr[:, :],
                                            axis=mybir.AxisListType.X, op=A_.add)
                    if DGDUMP:
                        nc.sync.dma_start(out=dg_out[:, k:k + 1], in_=dg[:, :])
                    rcp = S.tile([128, 1], dt.float32, tag="rcp", name="rcp")
                    nc.vector.reciprocal(out=rcp[:, :], in_=dg[:, :])
                    X = S.tile([128, 128], dt.float32, tag="Xns", name="Xns")
                    nc.vector.tensor_scalar(out=X[:, :], in0=Imask[:, :], scalar1=rcp[:, :],
                                            scalar2=None, op0=A_.mult)
                    for it in range(NS_ITERS):
                        pT = PQ.tile([128, 128], dt.float32, tag="psq", name="psq")
                        nc.tensor.matmul(pT[:, :], Dk[:, :], X[:, :], start=True, stop=True)
                        Z = S.tile([128, 128], dt.float32, tag="Zns", name="Zns")
                        nc.vector.scalar_tensor_tensor(out=Z[:, :], in0=Imask[:, :], scalar=2.0,
                                                       in1=pT[:, :], op0=A_.mult, op1=A_.subtract)
                        pX = PQ.tile([128, 128], dt.float32, tag="psq", name="psq")
                        nc.tensor.matmul(pX[:, :], X[:, :], Z[:, :], start=True, stop=True)
                        X = S.tile([128, 128], dt.float32, tag="Xns", name="Xns")
                        nc.vector.tensor_copy(out=X[:, :], in_=pX[:, :])
                    INVb = S.tile([128, 128], dt.bfloat16, tag="INVb", name="INVb")
                    nc.vector.tensor_copy(out=INVb[:, :], in_=X[:, :])

                    # V over local slab + border
                    pv = PW.tile([128, WSL], dt.float32, tag="pw", name="pw")
                    nc.tensor.matmul(pv[:, 0:512], INVb[:, :], rloc[:, :], start=True, stop=True)
                    nc.vector.tensor_copy(out=Vloc[kk][:, :], in_=pv[:, 0:512])
                    nc.tensor.matmul(pv[:, 512:WSL], INVb[:, :], rowbb[kk][:, :], start=True, stop=True)
                    nc.vector.tensor_copy(out=Vbord[kk][:, :], in_=pv[:, 512:WSL])

                    # V over full gathered width (for panel-row updates)
                    Vf = Vfull[kk % 2]
                    for n in range(8):
                        pf = PW.tile([128, 512], dt.float32, tag="pw", name="pw")
                        nc.tensor.matmul(pf[:, :], INVb[:, :], grow[kk][:, 512 * n:512 * n + 512],
                                         start=True, stop=True)
                        nc.vector.tensor_copy(out=Vf[:, 512 * n:512 * n + 512], in_=pf[:, :])

                    # within-panel updates of later pivot rows
                    for j in range(kk + 1, 4):
                        Gj = grow[kk][:, f_off(4 * s + j):f_off(4 * s + j) + 128]
                        for n in range(8):
                            pu = PW.tile([128, 512], dt.float32, tag="pw", name="pw")
                            nc.tensor.matmul(pu[:, :], Gj, Vf[:, 512 * n:512 * n + 512],
                                             start=True, stop=True)
                            nc.vector.scalar_tensor_tensor(
                                out=grow[j][:, 512 * n:512 * n + 512],
                                in0=grow[j][:, 512 * n:512 * n + 512], scalar=1.0,
                                in1=pu[:, :], op0=A_.mult, op1=A_.subtract)
                        # local slab + border of later pivot rows
                        rj = 4 * s + j
                        pl = PW.tile([128, WSL], dt.float32, tag="pw", name="pw")
                        nc.tensor.matmul(pl[:, 0:512], Gj, Vloc[kk][:, :], start=True, stop=True)
                        nc.tensor.matmul(pl[:, 512:WSL], Gj, Vbord[kk][:, :], start=True, stop=True)
                        nc.vector.scalar_tensor_tensor(out=Arow[rj][:, 0:512],
                                                       in0=Arow[rj][:, 0:512], scalar=1.0,
                                                       in1=pl[:, 0:512], op0=A_.mult, op1=A_.subtract)
                        nc.vector.scalar_tensor_tensor(out=Arow[rj][:, 512:WSL],
                                                       in0=Arow[rj][:, 512:WSL], scalar=1.0,
                                                       in1=pl[:, 512:WSL], op0=A_.mult, op1=A_.subtract)

                    # logdet pieces
                    if k >= 16:
                        lg = S.tile([128, 1], dt.float32, tag="lg", name="lg")
                        nc.scalar.activation(out=lg[:, :], in_=dg[:, :], func=AF.Ln)
                        sq = S.tile([128, 1], dt.float32, tag="sq", name="sq")
                        nc.scalar.activation(out=sq[:, :], in_=rcp[:, :], func=AF.Sqrt)
                        T1 = S.tile([128, 128], dt.float32, tag="T1", name="T1")
                        nc.vector.tensor_scalar(out=T1[:, :], in0=Dk[:, :], scalar1=sq[:, :],
                                                scalar2=None, op0=A_.mult)
                        psr = PT.tile([1, 128], dt.float32, tag="pt", name="pt")
                        nc.tensor.matmul(psr[:, :], sq[:, :], Imask[:, :], start=True, stop=True)
                        sqr = S.tile([1, 128], dt.float32, tag="sqr", name="sqr")
                        nc.vector.tensor_copy(out=sqr[:, :], in_=psr[:, :])
                        pbc = PQ.tile([128, 128], dt.float32, tag="psq", name="psq")
                        nc.tensor.matmul(pbc[:, :], onesr[:, :], sqr[:, :], start=True, stop=True)
                        E1 = S.tile([128, 128], dt.float32, tag="E1", name="E1")
                        nc.vector.scalar_tensor_tensor(out=E1[:, :], in0=T1[:, :], scalar=1.0,
                                                       in1=pbc[:, :], op0=A_.mult, op1=A_.mult)
                        E = S.tile([128, 128], dt.float32, tag="Emat", name="Emat")
                        nc.vector.scalar_tensor_tensor(out=E[:, :], in0=E1[:, :], scalar=1.0,
                                                       in1=Imask[:, :], op0=A_.mult, op1=A_.subtract)
                        pows = [E]
                        for (la, lb) in [(0, 0), (1, 0), (1, 1), (3, 0)]:
                            pp = PQ.tile([128, 128], dt.float32, tag="psq", name="psq")
                            nc.tensor.matmul(pp[:, :], pows[la][:, :], pows[lb][:, :],
                                             start=True, stop=True)
                            Ei = S.tile([128, 128], dt.float32, tag=f"E{len(pows) + 1}", name=f"E{len(pows) + 1}")
                            nc.vector.tensor_copy(out=Ei[:, :], in_=pp[:, :])
                            pows.append(Ei)
                        E2, E3, E4, E5 = pows[1], pows[2], pows[3], pows[4]
                        pairs = [(E, Imask, 1), (E, E, 2), (E2, E, 3), (E2, E2, 4), (E3, E2, 5),
                                 (E3, E3, 6), (E4, E3, 7), (E4, E4, 8), (E5, E4, 9), (E5, E5, 10)]
                        ser = None
                        for (Pa, Pb, order) in pairs:
                            scr2 = S.tile([128, 128], dt.float32, tag="scr2", name="scr2")
                            nc.vector.scalar_tensor_tensor(out=scr2[:, :], in0=Pa[:, :], scalar=1.0,
                                                           in1=Pb[:, :], op0=A_.mult, op1=A_.mult)
                            tr = S.tile([128, 1], dt.float32, tag=f"tr{order}", name=f"tr{order}")
                            nc.vector.tensor_reduce(out=tr[:, :], in_=scr2[:, :],
                                                    axis=mybir.AxisListType.X, op=A_.add)
                            coef = ((-1.0) ** (order + 1)) / order
                            if ser is None:
                                ser = S.tile([128, 1], dt.float32, tag="ser", name="ser")
                                nc.vector.tensor_scalar(out=ser[:, :], in0=tr[:, :], scalar1=coef,
                                                        scalar2=None, op0=A_.mult)
                            else:
                                ser2 = S.tile([128, 1], dt.float32, tag="ser", name="ser")
                                nc.vector.scalar_tensor_tensor(out=ser2[:, :], in0=tr[:, :],
                                                               scalar=coef, in1=ser[:, :],
                                                               op0=A_.mult, op1=A_.add)
                                ser = ser2
                        tot = S.tile([128, 1], dt.float32, tag="totld", name="totld")
                        nc.vector.scalar_tensor_tensor(out=tot[:, :], in0=lg[:, :], scalar=1.0,
                                                       in1=ser[:, :], op0=A_.mult, op1=A_.add)
                        nxt = 1 - ld_cur
                        nc.vector.scalar_tensor_tensor(out=ld_acc[nxt][:, :], in0=tot[:, :],
                                                       scalar=1.0, in1=ld_acc[ld_cur][:, :],
                                                       op0=A_.mult, op1=A_.add)
                        ld_cur = nxt

                # ---- batch update: next panel rows first ----
                nxt_rows = [r for r in range(4 * s + 4, min(4 * s + 8, NBT + 1))]
                batch_update(nxt_rows, Vloc, Vbord, rowbb, grow)

                # ---- gather next panel (overlaps remaining batch updates) ----
                if s + 1 < SSTEPS and nxt_rows and nxt_rows[0] + 3 < NBT + 1:
                    cinb = DR.tile([512, 512], dt.bfloat16, tag="cinb", name="cinb")
                    cout = DR.tile([4096, 512], dt.bfloat16, tag="coutb", name="coutb",
                                   addr_space="Local" if NOCC else "Shared")
                    for j in range(4):
                        rr = 4 * s + 4 + j
                        cb = S.tile([128, 512], dt.bfloat16, tag="cb", name="cb", bufs=4)
                        nc.vector.tensor_copy(out=cb[:, :], in_=Arow[rr][:, 0:512])
                        nc.gpsimd.dma_start(out=cinb[128 * j:128 * j + 128, :], in_=cb[:, :])
                    if NOCC:
                        for c in range(NCORES):
                            nc.gpsimd.dma_start(out=cout[512 * c:512 * c + 512, :], in_=cinb[:, :])
                    else:
                        nc.gpsimd.collective_compute("AllGather", A_.bypass,
                                                     replica_groups=[list(range(NCORES))],
                                                     ins=[cinb[:, :].opt()], outs=[cout[:, :].opt()])

                # ---- batch update: remaining rows ----
                rem_rows = [r for r in range(4 * s + 8, NBT + 1)]
                batch_update(rem_rows, Vloc, Vbord, rowbb, grow)

                if s == 3:
                    nc.vector.tensor_copy(out=cmid[:, :], in_=Arow[NBT][0:1, 512:513])
                if s == SSTEPS - 1:
                    nc.vector.tensor_copy(out=cend[:, :], in_=Arow[NBT][0:1, 512:513])

                # ---- DMA gathered panel into grow for next superstep ----
                if s + 1 < SSTEPS:
                    for kk in range(4):
                        for c in range(NCORES):
                            nc.gpsimd.dma_start(
                                out=grow[kk][:, 512 * c:512 * c + 512],
                                in_=cout[512 * c + 128 * kk:512 * c + 128 * kk + 128, :])

            # ---------- finale ----------
            pld = PT.tile([1, 1], dt.float32, tag="pt", name="pt")
            nc.tensor.matmul(pld[:, :], ld_acc[ld_cur][:, :], onesc[:, :], start=True, stop=True)
            ldsum = S.tile([1, 1], dt.float32, tag="ldsum", name="ldsum")
            nc.vector.tensor_copy(out=ldsum[:, :], in_=pld[:, :])
            qd = S.tile([1, 1], dt.float32, tag="qd", name="qd")
            nc.vector.scalar_tensor_tensor(out=qd[:, :], in0=cmid[:, :], scalar=1.0,
                                           in1=cend[:, :], op0=A_.mult, op1=A_.subtract)
            b1 = S.tile([1, 1], dt.float32, tag="b1", name="b1")
            nc.vector.tensor_scalar(out=b1[:, :], in0=qd[:, :], scalar1=0.5,
                                    scalar2=CONST, op0=A_.mult, op1=A_.add)
            lossv = S.tile([1, 1], dt.float32, tag="lossv", name="lossv")
            nc.scalar.activation(out=lossv[:, :], in_=ldsum[:, :], func=AF.Identity,
                                 bias=b1[:, :], scale=0.25)
            nc.sync.dma_start(out=loss_out[:, :], in_=lossv[:, :])
            st2.__exit__(None, None, None)

    nc.compile()
    _NC_CACHE = nc
    return nc


LAST_EXEC_NS = None

def kernel(**inputs):
    global LAST_EXEC_NS
    from concourse.bass_utils import run_bass_kernel_spmd
    f32 = np.float32

    def arr(x):
        return np.ascontiguousarray(np.asarray(x, dtype=f32))

    sx, tx = arr(inputs["source_x"]), arr(inputs["target_x"])
    sy, ty = arr(inputs["source_y"]), arr(inputs["target_y"])
    k_ss, k_tt, k_st = arr(inputs["k_ss"]), arr(inputs["k_tt"]), arr(inputs["k_st"])
    Wf, bf = arr(inputs["Wf"]), arr(inputs["bf"])
    Ws, bs = arr(inputs["Ws"]), arr(inputs["bs"])
    Wt, bt = arr(inputs["Wt"]), arr(inputs["bt"])
    Kb = arr(inputs["Kb"])
    base_s, base_t = arr(inputs["base_s"]), arr(inputs["base_t"])
    noise_s, noise_t = arr(inputs["noise_s_opt"]), arr(inputs["noise_t_opt"])

    assert np.all(bf == 0.0), "kernel assumes bf == 0"
    ybcol = np.concatenate([sy[:, 0], ty[:, 0], np.zeros(128, f32)]).reshape(4224, 1).astype(f32)
    ones_row = np.ones((1, 128), f32)
    Imask = np.eye(128, dtype=f32)
    onescol = np.ones((128, 1), f32)
    vaS = np.ascontiguousarray((Wf.T @ Ws[0, 0:8]).reshape(256, 1).astype(f32))
    vaT = np.ascontiguousarray((Wf.T @ Wt[0, 0:8]).reshape(256, 1).astype(f32))
    vcS = np.ascontiguousarray((Wf.T @ Ws[0, 8:16]).reshape(256, 1).astype(f32))
    vcT = np.ascontiguousarray((Wf.T @ Wt[0, 8:16]).reshape(256, 1).astype(f32))

    # khad for panel-0 rows (source rows 0..512) x all gathered columns
    khp0 = np.empty((512, 4096), f32)
    for c in range(NCORES):
        khp0[:, 512 * c:512 * c + 256] = k_ss[0:512, 256 * c:256 * c + 256]
        khp0[:, 512 * c + 256:512 * c + 512] = k_st[0:512, 256 * c:256 * c + 256]

    in_maps = []
    for c in range(NCORES):
        s0 = 256 * c
        nsel = [np.zeros((128, 16), f32) for _ in range(4)]
        nsel[0][:, 2 * c] = 1.0
        nsel[1][:, 2 * c + 1] = 1.0
        nsel[2][:, 2 * c] = 1.0
        nsel[3][:, 2 * c + 1] = 1.0
        khad_c = np.empty((4096, 512), f32)
        khad_c[0:2048, 0:256] = k_ss[:, s0:s0 + 256]
        khad_c[0:2048, 256:512] = k_st[:, s0:s0 + 256]
        khad_c[2048:4096, 0:256] = k_st[s0:s0 + 256, :].T
        khad_c[2048:4096, 256:512] = k_tt[:, s0:s0 + 256]
        yrow = np.zeros((1, WSL), f32)
        yrow[0, 0:256] = sy[s0:s0 + 256, 0]
        yrow[0, 256:512] = ty[s0:s0 + 256, 0]
        in_maps.append(dict(
            xsT=np.ascontiguousarray(sx[s0:s0 + 256, :].T),
            xtT=np.ascontiguousarray(tx[s0:s0 + 256, :].T),
            bsT=np.ascontiguousarray(base_s.T),
            btT=np.ascontiguousarray(base_t.T),
            vaS=vaS, vaT=vaT, vcS=vcS, vcT=vcT,
            bS=bs.reshape(1, 1), bT=bt.reshape(1, 1),
            noiseS=noise_s.reshape(1, 1), noiseT=noise_t.reshape(1, 1),
            Kb=Kb, khad=khad_c, khp0=khp0, ybcol=ybcol, yrow=yrow,
            ones_row=ones_row, Imask=Imask, onescol=onescol,
            nsel0=nsel[0], nsel1=nsel[1], nsel2=nsel[2], nsel3=nsel[3],
        ))

    nc = build()
    trace = bool(int(os.environ.get("KERNEL_TRACE", "0")))
    loss = None
    if os.environ.get("KERNEL_FORCE_SIM", "0") != "1":
        for attempt in range(2):
            try:
                res = run_bass_kernel_spmd(nc, in_maps, core_ids=list(range(NCORES)), trace=trace)
                LAST_EXEC_NS = res.exec_time_ns
                loss = np.float32(res.results[0]["loss"][0, 0])
                if np.isfinite(loss):
                    break
            except Exception as e:
                if os.environ.get("KERNEL_DEBUG", "0") == "1":
                    import traceback
                    traceback.print_exc()
                sys.stderr.write("HW attempt %d failed (%s)\n" % (attempt, type(e).__name__))
        else:
            sys.stderr.write("HW path failed; falling back to MultiCoreSim\n")
    if loss is None or not np.isfinite(loss):
        from concourse.bass_interp import MultiCoreSim
        sim = MultiCoreSim(nc, num_cores=NCORES, trace=False,
                           require_finite=False, require_nnan=False)
        for i in range(NCORES):
            for kk, vv in in_maps[i].items():
                sim.cores[i].tensor(kk)[:] = vv
        sim.simulate(check_with_hw=False)
        loss = np.float32(sim.cores[0].mem_tensor("loss")[0, 0])
    return np.asarray(loss, dtype=np.float32).reshape(())

